# revision 1
# baseline (speedup 1.0000x reference)
"""DKVMN scatter_memory kernel for 8 Trainium2 NeuronCores.

Math: the reference scan only ever uses the (B, M, Dv) memory through
read @ Wf_r, so the whole recurrence collapses to a 32-dim linear
cumulative sum:

  S  = softmax(Eq @ Wa + ba)            (100 x 32)  per-vocab att rows
  cq = Eq @ Wf[:64] + bf                (100,)
  cv = Ev @ Wf[64:]                     (100,)
  w  = (2q + a) % 100
  pred[t,b] = cq[q[t,b]] + sum_{s<t} cv[w[s,b]] * <S[q[t,b]], S[q[s,b]]>

Per core (batch-sharded, Bs=128): att rows are delivered by one-hot
matmuls on TensorE (one-hot built by DVE is_equal on a DMA-replicated
index row); the cumsum over t is a strict-lower-triangular matmul.
Layout: t on partitions, (b, m) on free dim.
"""
import functools
import numpy as np

import concourse.bass as bass
import concourse.bacc as bacc
import concourse.mybir as mybir
from concourse import tile
from concourse.bass_utils import run_bass_kernel_spmd

T, B, M, DQ, DV, VOCAB = 128, 1024, 32, 64, 64, 100
NCORES = 8
BS = B // NCORES  # 128
N = T * BS        # tokens per core = 16384
NG = 8            # b-groups
GB = BS // NG     # 16 b per group
F32 = mybir.dt.float32
F16 = mybir.dt.float16
I32 = mybir.dt.int32
AX = mybir.AxisListType
OP = mybir.AluOpType


def _build():
    nc = bacc.Bacc("TRN2", num_devices=NCORES, debug=False, target_bir_lowering=False)
    d = {}
    d["qT"] = nc.dram_tensor("qT", [BS, T], I32, kind="ExternalInput").ap()
    d["aT"] = nc.dram_tensor("aT", [BS, T], I32, kind="ExternalInput").ap()
    d["Eq"] = nc.dram_tensor("Eq", [VOCAB, DQ], F32, kind="ExternalInput").ap()
    d["Ev"] = nc.dram_tensor("Ev", [VOCAB, DV], F32, kind="ExternalInput").ap()
    d["Wa"] = nc.dram_tensor("Wa", [DQ, M], F32, kind="ExternalInput").ap()
    d["ba"] = nc.dram_tensor("ba", [1, M], F32, kind="ExternalInput").ap()
    d["Wf"] = nc.dram_tensor("Wf", [DQ + DV, 1], F32, kind="ExternalInput").ap()
    d["bf"] = nc.dram_tensor("bf", [1, 1], F32, kind="ExternalInput").ap()
    d["iota"] = nc.dram_tensor("iota", [128, 1], F32, kind="ExternalInput").ap()
    d["ident"] = nc.dram_tensor("ident", [128, 128], F32, kind="ExternalInput").ap()
    d["ustrict"] = nc.dram_tensor("ustrict", [128, 128], F32, kind="ExternalInput").ap()
    d["ones"] = nc.dram_tensor("ones", [1, 128], F32, kind="ExternalInput").ap()
    preds = nc.dram_tensor("preds", [T, BS], F32, kind="ExternalOutput").ap()

    with tile.TileContext(nc) as tc:
        with (
            tc.tile_pool(name="sb", bufs=1) as sb,
            tc.tile_pool(name="ps", bufs=2, space="PSUM") as ps,
        ):
            # ---- loads ----
            eq_t = sb.tile([VOCAB, DQ], F32)
            ev_t = sb.tile([VOCAB, DV], F32)
            wa_t = sb.tile([DQ, M], F32)
            ba_t = sb.tile([1, M], F32)
            wf_t = sb.tile([DQ + DV, 1], F32)
            bf_t = sb.tile([1, 1], F32)
            io_t = sb.tile([128, 1], F32)
            id_t = sb.tile([128, 128], F32)
            us_t = sb.tile([128, 128], F16)
            usf_t = sb.tile([128, 128], F32)
            on_t = sb.tile([1, 128], F32)
            qT_t = sb.tile([BS, T], I32)
            aT_t = sb.tile([BS, T], I32)
            for name, t_ in (("Eq", eq_t), ("Ev", ev_t), ("Wa", wa_t), ("ba", ba_t),
                             ("Wf", wf_t), ("bf", bf_t), ("iota", io_t),
                             ("ident", id_t), ("ustrict", usf_t), ("ones", on_t),
                             ("qT", qT_t), ("aT", aT_t)):
                nc.sync.dma_start(t_[:], d[name][:])

            nc.vector.tensor_copy(us_t[:], usf_t[:])
            # ---- index prep: w = (2q + a) % 100 on (b x t) ----
            w_t = sb.tile([BS, T], I32)
            m_t = sb.tile([BS, T], I32)
            nc.vector.tensor_scalar_mul(w_t[:], qT_t[:], 2)
            nc.vector.tensor_add(w_t[:], w_t[:], aT_t[:])
            # subtract 200 if >= 200
            nc.vector.tensor_scalar(out=m_t[:], in0=w_t[:], scalar1=200,
                                    scalar2=None, op0=OP.is_ge)
            nc.vector.tensor_scalar_mul(m_t[:], m_t[:], 200)
            nc.vector.tensor_tensor(w_t[:], w_t[:], m_t[:], OP.subtract)
            # subtract 100 if >= 100
            nc.vector.tensor_scalar(out=m_t[:], in0=w_t[:], scalar1=100,
                                    scalar2=None, op0=OP.is_ge)
            nc.vector.tensor_scalar_mul(m_t[:], m_t[:], 100)
            nc.vector.tensor_tensor(w_t[:], w_t[:], m_t[:], OP.subtract)

            qf32_t = sb.tile([BS, T], F32)
            wf32_t = sb.tile([BS, T], F32)
            qf_t = sb.tile([BS, T], F16)
            wf16_t = sb.tile([BS, T], F16)
            nc.vector.tensor_copy(qf32_t[:], qT_t[:])
            nc.vector.tensor_copy(wf32_t[:], w_t[:])
            nc.vector.tensor_copy(qf_t[:], qf32_t[:])
            nc.vector.tensor_copy(wf16_t[:], wf32_t[:])

            # ---- combo row (1 x 2N) then replicate to VOCAB partitions ----
            repl = sb.tile([VOCAB, 2 * N], F16)
            nc.sync.dma_start(repl[0:1, 0:N], qf_t[:])
            nc.sync.dma_start(repl[0:1, N:2 * N], wf16_t[:])
            CW = (2 * N) // 4
            k = 1
            while k < VOCAB:
                n = min(k, VOCAB - k)
                for ch in range(4):
                    eng = nc.sync if ch % 2 == 0 else nc.scalar
                    eng.dma_start(repl[k:k + n, ch * CW:(ch + 1) * CW],
                                  repl[0:n, ch * CW:(ch + 1) * CW])
                k += n

            # ---- parameter tables ----
            # EqT / EvT via PE transpose
            p_eqT = ps.tile([DQ, 128], F32, tag="pA")
            p_evT = ps.tile([DV, 128], F32, tag="pR")
            eqT_t = sb.tile([DQ, VOCAB], F32)
            evT_t = sb.tile([DV, VOCAB], F32)
            nc.tensor.transpose(p_eqT[:, 0:VOCAB], eq_t[:], id_t[0:VOCAB, 0:VOCAB])
            nc.scalar.copy(eqT_t[:], p_eqT[:, 0:VOCAB])
            nc.tensor.transpose(p_evT[:, 0:VOCAB], ev_t[:], id_t[0:VOCAB, 0:VOCAB])
            nc.scalar.copy(evT_t[:], p_evT[:, 0:VOCAB])

            # S = softmax(Eq@Wa + ba) -> fp16
            p_s = ps.tile([VOCAB, M], F32, tag="pC")
            nc.tensor.matmul(p_s[:], eqT_t[:], wa_t[:], start=True, stop=False)
            nc.tensor.matmul(p_s[:], on_t[0:1, 0:VOCAB], ba_t[:], start=False, stop=True)
            mx_t = sb.tile([VOCAB, 1], F32)
            sm_t = sb.tile([VOCAB, 1], F32)
            se_t = sb.tile([VOCAB, M], F32)
            s16_t = sb.tile([VOCAB, M], F16)
            nc.vector.tensor_reduce(mx_t[:], p_s[:], AX.X, OP.max)
            nc.vector.tensor_scalar_mul(mx_t[:], mx_t[:], -1.0)
            nc.scalar.activation(se_t[:], p_s[:],
                                 mybir.ActivationFunctionType.Exp,
                                 bias=mx_t[:], scale=1.0)
            nc.vector.tensor_reduce(sm_t[:], se_t[:], AX.X, OP.add)
            nc.vector.reciprocal(sm_t[:], sm_t[:])
            nc.vector.tensor_scalar(out=s16_t[:], in0=se_t[:], scalar1=sm_t[:],
                                    scalar2=None, op0=OP.mult)

            # cq = Eq@Wf_q + bf (100x1) fp16 ; cvr = Ev @ (Wf_r repl 32) fp16
            p_cq = ps.tile([VOCAB, 1], F32, tag="pP")
            nc.tensor.matmul(p_cq[:], eqT_t[:], wf_t[0:DQ, :], start=True, stop=False)
            nc.tensor.matmul(p_cq[:], on_t[0:1, 0:VOCAB], bf_t[:], start=False, stop=True)
            cq16_t = sb.tile([VOCAB, 1], F16)
            nc.scalar.copy(cq16_t[:], p_cq[:])
            wfr_t = sb.tile([DV, M], F32)
            nc.vector.tensor_scalar(out=wfr_t[:], in0=id_t[0:DV, 0:M], scalar1=0.0,
                                    scalar2=wf_t[DQ:DQ + DV, :], op0=OP.mult,
                                    op1=OP.add)
            p_cvr = ps.tile([VOCAB, M], F32, tag="pA")
            nc.tensor.matmul(p_cvr[:], evT_t[:], wfr_t[:], start=True, stop=True)
            cvr16_t = sb.tile([VOCAB, M], F16)
            nc.scalar.copy(cvr16_t[:], p_cvr[:])

            # ---- one-hots ----
            oh = sb.tile([VOCAB, 2 * N], F16)
            # ---- main pipeline ----
            a_sb = sb.tile([128, 512], F32, tag="a_sb")
            v_sb = sb.tile([128, 512], F16, tag="v_sb")
            ap_sb = sb.tile([128, 512], F32, tag="ap_sb")
            c_sb = sb.tile([128, BS], F32)
            out_sb = sb.tile([128, BS], F32)

            for g in range(NG):
                sl_q = slice(g * GB * T, (g + 1) * GB * T)
                sl_w = slice(N + g * GB * T, N + (g + 1) * GB * T)
                nc.vector.tensor_scalar(out=oh[:, sl_q], in0=repl[:, sl_q],
                                        scalar1=io_t[0:VOCAB, :], scalar2=None,
                                        op0=OP.is_equal)
                nc.vector.tensor_scalar(out=oh[:, sl_w], in0=repl[:, sl_w],
                                        scalar1=io_t[0:VOCAB, :], scalar2=None,
                                        op0=OP.is_equal)
                pA = ps.tile([128, 512], F32, tag="pA")
                pR = ps.tile([128, 512], F32, tag="pR")
                pC = ps.tile([128, GB], F32, tag="pC")
                pP = ps.tile([128, 512], F32, tag="pP")
                a_g = sb.tile([128, 512], F32, tag="a_sb")
                v_g = sb.tile([128, 512], F16, tag="v_sb")
                ap_g = sb.tile([128, 512], F32, tag="ap_sb")
                for k in range(GB):
                    tok = (g * GB + k) * T
                    ohq = oh[:, tok:tok + T]
                    ohw = oh[:, N + tok:N + tok + T]
                    nc.tensor.matmul(pA[:, k * M:(k + 1) * M], ohq, s16_t[:],
                                     start=True, stop=True)
                    nc.tensor.matmul(pC[:, k:k + 1], ohq, cq16_t[:],
                                     start=True, stop=True)
                    nc.tensor.matmul(pR[:, k * M:(k + 1) * M], ohw, cvr16_t[:],
                                     start=True, stop=True)
                nc.scalar.copy(a_g[:], pA[:])
                nc.scalar.copy(c_sb[:, g * GB:(g + 1) * GB], pC[:])
                nc.vector.tensor_tensor(v_g[:], a_g[:], pR[:], OP.mult)
                nc.tensor.matmul(pP[:], us_t[:], v_g[:], start=True, stop=True)
                nc.vector.tensor_tensor(ap_g[:], a_g[:], pP[:], OP.mult)
                nc.vector.tensor_reduce(
                    out_sb[:, g * GB:(g + 1) * GB],
                    ap_g[:].rearrange("p (b m) -> p b m", m=M),
                    AX.X, OP.add)

            nc.vector.tensor_add(out_sb[:], out_sb[:], c_sb[:])
            nc.sync.dma_start(preds[:], out_sb[:])

    nc.compile()
    return nc


@functools.lru_cache(maxsize=1)
def _get_nc():
    return _build()


def kernel(questions, answers, Eq, Ev, Wa, ba, Wf, bf):
    questions = np.asarray(questions)
    answers = np.asarray(answers)
    consts = {
        "Eq": np.asarray(Eq, np.float32),
        "Ev": np.asarray(Ev, np.float32),
        "Wa": np.asarray(Wa, np.float32),
        "ba": np.asarray(ba, np.float32).reshape(1, M),
        "Wf": np.asarray(Wf, np.float32).reshape(DQ + DV, 1),
        "bf": np.asarray(bf, np.float32).reshape(1, 1),
        "iota": np.arange(128, dtype=np.float32).reshape(128, 1),
        "ident": np.eye(128, dtype=np.float32),
        "ustrict": np.triu(np.ones((128, 128), np.float32), k=1),
        "ones": np.ones((1, 128), np.float32),
    }
    nc = _get_nc()
    in_maps = []
    for c in range(NCORES):
        sl = slice(c * BS, (c + 1) * BS)
        m = dict(consts)
        m["qT"] = np.ascontiguousarray(questions[:, sl].T).astype(np.int32)
        m["aT"] = np.ascontiguousarray(answers[:, sl].T).astype(np.int32)
        in_maps.append(m)
    res = run_bass_kernel_spmd(nc, in_maps, list(range(NCORES)))
    preds = np.concatenate([res.results[c]["preds"] for c in range(NCORES)], axis=1)
    return preds.astype(np.float32)



# revision 6
# speedup vs baseline: 1.8548x; 1.8548x over previous
"""DKVMN scatter_memory kernel for 8 Trainium2 NeuronCores.

Math: the reference scan only ever uses the (B, M, Dv) memory through
read @ Wf_r, so the whole recurrence collapses to a 32-dim linear
cumulative sum:

  S  = softmax(Eq @ Wa + ba)            (100 x 32)  per-vocab att rows
  cq = Eq @ Wf[:64] + bf                (100,)
  cv = Ev @ Wf[64:]                     (100,)
  w  = (2q + a) % 100
  pred[t,b] = cq[q[t,b]] + sum_{s<t} cv[w[s,b]] * <S[q[t,b]], S[q[s,b]]>

Per core (batch-sharded, Bs=128): the host precomputes one-hot
encodings of q and w (pure index preprocessing); the device gathers
S-rows / cq / cv by one-hot matmuls on TensorE ([S|cq] streamed in one
33-col matmul per batch element), then the cumsum over t is a strict-
lower-triangular matmul.  Layout: t on partitions, (b, m) on free dim.
"""
import functools
import numpy as np

import concourse.bass as bass
import concourse.bacc as bacc
import concourse.mybir as mybir
from concourse import tile
from concourse.bass_utils import run_bass_kernel_spmd

T, B, M, DQ, DV, VOCAB = 128, 1024, 32, 64, 64, 100
NCORES = 8
BS = B // NCORES  # 128
N = T * BS        # tokens per core = 16384
GB = 15           # b per group (33*15=495 <= 512 psum bank)
GROUPS = [(g * GB, GB) for g in range(8)] + [(120, 8)]
F32 = mybir.dt.float32
F16 = mybir.dt.float16
AX = mybir.AxisListType
OP = mybir.AluOpType


def _build():
    nc = bacc.Bacc("TRN2", num_devices=NCORES, debug=False, target_bir_lowering=False)
    d = {}
    d["Eq"] = nc.dram_tensor("Eq", [VOCAB, DQ], F32, kind="ExternalInput").ap()
    d["Ev"] = nc.dram_tensor("Ev", [VOCAB, DV], F32, kind="ExternalInput").ap()
    d["Wa"] = nc.dram_tensor("Wa", [DQ, M], F32, kind="ExternalInput").ap()
    d["ba"] = nc.dram_tensor("ba", [1, M], F32, kind="ExternalInput").ap()
    d["Wf"] = nc.dram_tensor("Wf", [DQ + DV, 1], F32, kind="ExternalInput").ap()
    d["bf"] = nc.dram_tensor("bf", [1, 1], F32, kind="ExternalInput").ap()
    d["ident"] = nc.dram_tensor("ident", [VOCAB, VOCAB], F32, kind="ExternalInput").ap()
    d["ones"] = nc.dram_tensor("ones", [1, VOCAB], F32, kind="ExternalInput").ap()
    d["us16"] = nc.dram_tensor("us16", [128, 128], F16, kind="ExternalInput").ap()
    d["ohq"] = nc.dram_tensor("ohq", [VOCAB, N], F16, kind="ExternalInput").ap()
    d["ohw"] = nc.dram_tensor("ohw", [VOCAB, N], F16, kind="ExternalInput").ap()
    preds = nc.dram_tensor("preds", [T, BS], F32, kind="ExternalOutput").ap()

    with tile.TileContext(nc) as tc:
        with (
            tc.tile_pool(name="sb", bufs=1) as sb,
            tc.tile_pool(name="ps", bufs=2, space="PSUM") as ps,
        ):
            # ---- param loads (scalar queue) ----
            eq_t = sb.tile([VOCAB, DQ], F32)
            ev_t = sb.tile([VOCAB, DV], F32)
            wa_t = sb.tile([DQ, M], F32)
            ba_t = sb.tile([1, M], F32)
            wf_t = sb.tile([DQ + DV, 1], F32)
            bf_t = sb.tile([1, 1], F32)
            id_t = sb.tile([VOCAB, VOCAB], F32)
            on_t = sb.tile([1, VOCAB], F32)
            us_t = sb.tile([128, 128], F16)
            wfr_t = sb.tile([DV, 1], F32)
            nc.scalar.dma_start(wfr_t[:], d["Wf"][DQ:DQ + DV, :])
            for name, t_ in (("Eq", eq_t), ("Ev", ev_t), ("Wa", wa_t), ("ba", ba_t),
                             ("Wf", wf_t), ("bf", bf_t), ("ident", id_t),
                             ("ones", on_t), ("us16", us_t)):
                nc.scalar.dma_start(t_[:], d[name][:])

            # ---- one-hot chunk loads, round-robin across queues ----
            qtiles, wtiles = [], []
            engs = [nc.sync, nc.scalar, nc.gpsimd]
            ei = 0
            for (b0, gb) in GROUPS:
                qt = sb.tile([VOCAB, gb * T], F16)
                wt = sb.tile([VOCAB, gb * T], F16)
                engs[ei % 3].dma_start(qt[:], d["ohq"][:, b0 * T:(b0 + gb) * T])
                engs[(ei + 1) % 3].dma_start(wt[:], d["ohw"][:, b0 * T:(b0 + gb) * T])
                ei += 2
                qtiles.append(qt)
                wtiles.append(wt)

            # ---- parameter tables ----
            # EqT / EvT via PE transpose
            p_eqT = ps.tile([DQ, VOCAB], F32, tag="pA")
            p_evT = ps.tile([DV, VOCAB], F32, tag="pP")
            eqT_t = sb.tile([DQ, VOCAB], F32)
            evT_t = sb.tile([DV, VOCAB], F32)
            nc.tensor.transpose(p_eqT[:], eq_t[:], id_t[:])
            nc.scalar.copy(eqT_t[:], p_eqT[:])
            nc.tensor.transpose(p_evT[:], ev_t[:], id_t[:])
            nc.scalar.copy(evT_t[:], p_evT[:])

            # scat = [softmax(Eq@Wa+ba) | cq]  (100 x 33) fp16
            scat = sb.tile([VOCAB, M + 1], F16)
            p_s = ps.tile([VOCAB, M], F32, tag="pCV")
            nc.tensor.matmul(p_s[:], eqT_t[:], wa_t[:], start=True, stop=False)
            nc.tensor.matmul(p_s[:], on_t[:], ba_t[:], start=False, stop=True)
            mx_t = sb.tile([VOCAB, 1], F32)
            sm_t = sb.tile([VOCAB, 1], F32)
            se_t = sb.tile([VOCAB, M], F32)
            nc.vector.tensor_reduce(mx_t[:], p_s[:], AX.X, OP.max)
            nc.vector.tensor_scalar_mul(mx_t[:], mx_t[:], -1.0)
            nc.scalar.activation(se_t[:], p_s[:],
                                 mybir.ActivationFunctionType.Exp,
                                 bias=mx_t[:], scale=1.0)
            nc.vector.tensor_reduce(sm_t[:], se_t[:], AX.X, OP.add)
            nc.vector.reciprocal(sm_t[:], sm_t[:])
            nc.vector.tensor_scalar(out=scat[:, 0:M], in0=se_t[:], scalar1=sm_t[:],
                                    scalar2=None, op0=OP.mult)
            p_cq = ps.tile([VOCAB, 1], F32, tag="pA")
            nc.tensor.matmul(p_cq[:], eqT_t[:], wf_t[0:DQ, :], start=True, stop=False)
            nc.tensor.matmul(p_cq[:], on_t[:], bf_t[:], start=False, stop=True)
            nc.scalar.copy(scat[:, M:M + 1], p_cq[:])

            # cv = Ev @ Wf_r  (100 x 1) fp16
            cv16 = sb.tile([VOCAB, 1], F16)
            p_cv = ps.tile([VOCAB, 1], F32, tag="pP")
            nc.tensor.matmul(p_cv[:], evT_t[:], wfr_t[:], start=True, stop=True)
            nc.scalar.copy(cv16[:], p_cv[:])

            # ---- main pipeline ----
            out_sb = sb.tile([128, BS], F32)
            c_sb = sb.tile([128, BS], F32)

            for gi, (b0, gb) in enumerate(GROUPS):
                ohq_g = qtiles[gi]
                ohw_g = wtiles[gi]
                pA = ps.tile([128, gb * (M + 1)], F32, tag="pA")
                pCV = ps.tile([128, gb], F32, tag="pCV")
                pP = ps.tile([128, gb * M], F32, tag="pP")
                a_g = sb.tile([128, 512], F16, tag="a_sb")
                v_g = sb.tile([128, 512], F16, tag="v_sb")
                ap_g = sb.tile([128, 512], F16, tag="ap_sb")
                cvw_g = sb.tile([128, 16], F16, tag="cvw_sb")
                for k in range(gb):
                    nc.tensor.matmul(pA[:, k * (M + 1):(k + 1) * (M + 1)],
                                     ohq_g[:, k * T:(k + 1) * T], scat[:],
                                     start=True, stop=True)
                    nc.tensor.matmul(pCV[:, k:k + 1],
                                     ohw_g[:, k * T:(k + 1) * T], cv16[:],
                                     start=True, stop=True)
                # A rows (f16, strided source skipping the cq column)
                pA3 = pA[:].rearrange("p (k c) -> p k c", c=M + 1)
                nc.scalar.copy(a_g[:, 0:gb * M].rearrange("p (k c) -> p k c", c=M),
                               pA3[:, :, 0:M])
                # cq column -> c_sb
                nc.scalar.copy(c_sb[:, b0:b0 + gb], pA3[:, :, M:M + 1])
                # cvw (f16)
                nc.scalar.copy(cvw_g[:, 0:gb], pCV[:])
                # v = A * cv[w] (broadcast cvw along m)
                a3 = a_g[:, 0:gb * M].rearrange("p (k c) -> p k c", c=M)
                cvb = cvw_g[:, 0:gb].rearrange("p (k c) -> p k c", c=1)
                a3b, cvb = bass.broadcast_tensor_aps(a3, cvb)
                nc.vector.tensor_tensor(
                    v_g[:, 0:gb * M].rearrange("p (k c) -> p k c", c=M),
                    a3b, cvb, OP.mult)
                # exclusive cumsum over t (strict upper as lhsT)
                nc.tensor.matmul(pP[:], us_t[:], v_g[:, 0:gb * M],
                                 start=True, stop=True)
                # pred contribution: sum_m A * C
                nc.vector.tensor_tensor(
                    ap_g[:, 0:gb * M], a_g[:, 0:gb * M], pP[:], OP.mult)
                nc.vector.tensor_reduce(
                    out_sb[:, b0:b0 + gb],
                    ap_g[:, 0:gb * M].rearrange("p (b m) -> p b m", m=M),
                    AX.X, OP.add)

            nc.vector.tensor_add(out_sb[:], out_sb[:], c_sb[:])
            nc.sync.dma_start(preds[:], out_sb[:])

    nc.compile()
    return nc


@functools.lru_cache(maxsize=1)
def _get_nc():
    return _build()


def _onehot(idx_flat: np.ndarray) -> np.ndarray:
    """[VOCAB, len(idx)] f16 one-hot, column j = e_{idx[j]}."""
    oh = np.zeros((VOCAB, idx_flat.shape[0]), dtype=np.float16)
    oh[idx_flat, np.arange(idx_flat.shape[0])] = np.float16(1.0)
    return oh


def _in_maps(questions, answers, Eq, Ev, Wa, ba, Wf, bf):
    questions = np.asarray(questions)
    answers = np.asarray(answers)
    w = (questions.astype(np.int64) * 2 + answers.astype(np.int64)) % VOCAB
    consts = {
        "Eq": np.asarray(Eq, np.float32),
        "Ev": np.asarray(Ev, np.float32),
        "Wa": np.asarray(Wa, np.float32),
        "ba": np.asarray(ba, np.float32).reshape(1, M),
        "Wf": np.asarray(Wf, np.float32).reshape(DQ + DV, 1),
        "bf": np.asarray(bf, np.float32).reshape(1, 1),
        "ident": np.eye(VOCAB, dtype=np.float32),
        "ones": np.ones((1, VOCAB), np.float32),
        "us16": np.triu(np.ones((128, 128), np.float16), k=1),
    }
    in_maps = []
    for c in range(NCORES):
        sl = slice(c * BS, (c + 1) * BS)
        m = dict(consts)
        # token order (b, t): column b*T + t
        m["ohq"] = _onehot(np.ascontiguousarray(questions[:, sl].T).ravel())
        m["ohw"] = _onehot(np.ascontiguousarray(w[:, sl].T).ravel())
        in_maps.append(m)
    return in_maps


def kernel(questions, answers, Eq, Ev, Wa, ba, Wf, bf):
    nc = _get_nc()
    in_maps = _in_maps(questions, answers, Eq, Ev, Wa, ba, Wf, bf)
    res = run_bass_kernel_spmd(nc, in_maps, list(range(NCORES)))
    preds = np.concatenate([res.results[c]["preds"] for c in range(NCORES)], axis=1)
    return preds.astype(np.float32)


# revision 9
# speedup vs baseline: 1.8596x; 1.0026x over previous
"""DKVMN scatter_memory kernel for 8 Trainium2 NeuronCores.

Math: the reference scan only ever uses the (B, M, Dv) memory through
read @ Wf_r, so the whole recurrence collapses to a 32-dim linear
cumulative sum:

  S  = softmax(Eq @ Wa + ba)            (100 x 32)  per-vocab att rows
  cq = Eq @ Wf[:64] + bf                (100,)
  cv = Ev @ Wf[64:]                     (100,)
  w  = (2q + a) % 100
  pred[t,b] = cq[q[t,b]] + sum_{s<t} cv[w[s,b]] * <S[q[t,b]], S[q[s,b]]>

Per core (batch-sharded, Bs=128): the host precomputes one-hot
encodings of q and w (pure index preprocessing); the device gathers
S-rows / cq / cv by one-hot matmuls on TensorE ([S|cq] streamed in one
33-col matmul per batch element), then the cumsum over t is a strict-
lower-triangular matmul.  Layout: t on partitions, (b, m) on free dim.
All parameters arrive in a single packed f32 tensor (one DMA); the
one-hots stream in three chunks per side on the two HWDGE queues.
"""
import functools
import numpy as np

import concourse.bass as bass
import concourse.bacc as bacc
import concourse.mybir as mybir
from concourse import tile
from concourse.bass_utils import run_bass_kernel_spmd

T, B, M, DQ, DV, VOCAB = 128, 1024, 32, 64, 64, 100
NCORES = 8
BS = B // NCORES  # 128
N = T * BS        # tokens per core = 16384
GB = 15           # b per group (33*15=495, +15 cv cols = 510 <= 512 psum bank)
GROUPS = [(g * GB, GB) for g in range(8)] + [(120, 8)]
CHUNKS = [(0, 45), (45, 45), (90, 38)]   # (first b, n_b) per DMA chunk
F32 = mybir.dt.float32
F16 = mybir.dt.float16
AX = mybir.AxisListType
OP = mybir.AluOpType

# packed-parameter column layout (f32 [128, PC])
_EQ, _EV, _WA, _WFQ, _WFR = 0, 64, 128, 160, 161
_ID, _US, _ONE, _BA, _BF = 162, 262, 390, 490, 522
PC = 523


def _build():
    nc = bacc.Bacc("TRN2", num_devices=NCORES, debug=False, target_bir_lowering=False)
    d = {}
    d["pack"] = nc.dram_tensor("pack", [128, PC], F32, kind="ExternalInput").ap()
    d["ohq"] = nc.dram_tensor("ohq", [VOCAB, N], F16, kind="ExternalInput").ap()
    d["ohw"] = nc.dram_tensor("ohw", [VOCAB, N], F16, kind="ExternalInput").ap()
    preds = nc.dram_tensor("preds", [T, BS], F32, kind="ExternalOutput").ap()

    with tile.TileContext(nc) as tc:
        with (
            tc.tile_pool(name="sb", bufs=1) as sb,
            tc.tile_pool(name="wk", bufs=3) as wk,
            tc.tile_pool(name="ps", bufs=3, space="PSUM") as ps,
        ):
            P = sb.tile([128, PC], F32)
            nc.scalar.dma_start(P[:], d["pack"][:])
            qtiles, wtiles = [], []
            for (cb0, cnb) in CHUNKS:
                qt = sb.tile([VOCAB, cnb * T], F16)
                wt = sb.tile([VOCAB, cnb * T], F16)
                nc.sync.dma_start(qt[:], d["ohq"][:, cb0 * T:(cb0 + cnb) * T])
                nc.scalar.dma_start(wt[:], d["ohw"][:, cb0 * T:(cb0 + cnb) * T])
                qtiles.append(qt)
                wtiles.append(wt)

            us_t = sb.tile([128, 128], F16)
            nc.vector.tensor_copy(us_t[:], P[:, _US:_US + 128])

            # ---- parameter tables ----
            p_eqT = ps.tile([DQ, VOCAB], F32, tag="pA")
            p_evT = ps.tile([DV, VOCAB], F32, tag="pP")
            eqT_t = sb.tile([DQ, VOCAB], F32)
            evT_t = sb.tile([DV, VOCAB], F32)
            nc.tensor.transpose(p_eqT[:], P[0:VOCAB, _EQ:_EQ + DQ], P[0:VOCAB, _ID:_ID + VOCAB])
            nc.scalar.copy(eqT_t[:], p_eqT[:])
            nc.tensor.transpose(p_evT[:], P[0:VOCAB, _EV:_EV + DV], P[0:VOCAB, _ID:_ID + VOCAB])
            nc.scalar.copy(evT_t[:], p_evT[:])

            # scat = [softmax(Eq@Wa+ba) | cq]  (100 x 33) fp16
            scat = sb.tile([VOCAB, M + 1], F16)
            p_s = ps.tile([VOCAB, M], F32, tag="pA")
            nc.tensor.matmul(p_s[:], eqT_t[:], P[0:DQ, _WA:_WA + M], start=True, stop=False)
            nc.tensor.matmul(p_s[:], P[0:1, _ONE:_ONE + VOCAB], P[0:1, _BA:_BA + M],
                             start=False, stop=True)
            mx_t = sb.tile([VOCAB, 1], F32)
            sm_t = sb.tile([VOCAB, 1], F32)
            se_t = sb.tile([VOCAB, M], F32)
            nc.vector.tensor_reduce(mx_t[:], p_s[:], AX.X, OP.max)
            nc.vector.tensor_scalar_mul(mx_t[:], mx_t[:], -1.0)
            nc.scalar.activation(se_t[:], p_s[:],
                                 mybir.ActivationFunctionType.Exp,
                                 bias=mx_t[:], scale=1.0)
            nc.vector.tensor_reduce(sm_t[:], se_t[:], AX.X, OP.add)
            nc.vector.reciprocal(sm_t[:], sm_t[:])
            nc.vector.tensor_scalar(out=scat[:, 0:M], in0=se_t[:], scalar1=sm_t[:],
                                    scalar2=None, op0=OP.mult)
            p_cq = ps.tile([VOCAB, 1], F32, tag="pP")
            nc.tensor.matmul(p_cq[:], eqT_t[:], P[0:DQ, _WFQ:_WFQ + 1], start=True, stop=False)
            nc.tensor.matmul(p_cq[:], P[0:1, _ONE:_ONE + VOCAB], P[0:1, _BF:_BF + 1],
                             start=False, stop=True)
            nc.scalar.copy(scat[:, M:M + 1], p_cq[:])

            # cv = Ev @ Wf_r  (100 x 1) fp16
            cv16 = sb.tile([VOCAB, 1], F16)
            p_cv = ps.tile([VOCAB, 1], F32, tag="pA")
            nc.tensor.matmul(p_cv[:], evT_t[:], P[0:DV, _WFR:_WFR + 1], start=True, stop=True)
            nc.scalar.copy(cv16[:], p_cv[:])

            # ---- main pipeline ----
            out_sb = sb.tile([128, BS], F32)
            c_sb = sb.tile([128, BS], F32)

            for gi, (b0, gb) in enumerate(GROUPS):
                ci = gi // 3
                cb0 = CHUNKS[ci][0]
                off = (b0 - cb0) * T
                ohq_g = qtiles[ci]
                ohw_g = wtiles[ci]
                pA = ps.tile([128, 510], F32, tag="pA")
                pP = ps.tile([128, 480], F32, tag="pP")
                a_g = wk.tile([128, 480], F16, tag="a_sb")
                v_g = wk.tile([128, 480], F16, tag="v_sb")
                ap_g = wk.tile([128, 480], F16, tag="ap_sb")
                cvw_g = wk.tile([128, GB], F16, tag="cvw_sb")
                for k in range(gb):
                    nc.tensor.matmul(pA[:, k * 33:k * 33 + 33],
                                     ohq_g[:, off + k * T:off + (k + 1) * T], scat[:],
                                     start=True, stop=True)
                    nc.tensor.matmul(pA[:, 495 + k:496 + k],
                                     ohw_g[:, off + k * T:off + (k + 1) * T], cv16[:],
                                     start=True, stop=True)
                pA3 = pA[:, 0:gb * 33].rearrange("p (k c) -> p k c", c=33)
                # cq column -> c_sb ; cv row -> f16 ; A rows -> f16
                nc.scalar.copy(c_sb[:, b0:b0 + gb], pA3[:, :, M:M + 1])
                nc.scalar.copy(cvw_g[:, 0:gb], pA[:, 495:495 + gb])
                nc.scalar.copy(a_g[:, 0:gb * M].rearrange("p (k c) -> p k c", c=M),
                               pA3[:, :, 0:M])
                # v = A * cv[w] (broadcast cvw along m)
                a3 = a_g[:, 0:gb * M].rearrange("p (k c) -> p k c", c=M)
                cvb = cvw_g[:, 0:gb].rearrange("p (k c) -> p k c", c=1)
                a3b, cvb = bass.broadcast_tensor_aps(a3, cvb)
                nc.vector.tensor_tensor(
                    v_g[:, 0:gb * M].rearrange("p (k c) -> p k c", c=M),
                    a3b, cvb, OP.mult)
                # exclusive cumsum over t (strict upper as lhsT)
                nc.tensor.matmul(pP[:, 0:gb * M], us_t[:], v_g[:, 0:gb * M],
                                 start=True, stop=True)
                # pred contribution: sum_m A * C
                nc.vector.tensor_tensor(
                    ap_g[:, 0:gb * M], a_g[:, 0:gb * M], pP[:, 0:gb * M], OP.mult)
                nc.vector.tensor_reduce(
                    out_sb[:, b0:b0 + gb],
                    ap_g[:, 0:gb * M].rearrange("p (b m) -> p b m", m=M),
                    AX.X, OP.add)

            nc.vector.tensor_add(out_sb[:], out_sb[:], c_sb[:])
            nc.sync.dma_start(preds[:], out_sb[:])

    nc.compile()
    return nc


@functools.lru_cache(maxsize=1)
def _get_nc():
    return _build()


def _onehot(idx_flat: np.ndarray) -> np.ndarray:
    """[VOCAB, len(idx)] f16 one-hot, column j = e_{idx[j]}."""
    oh = np.zeros((VOCAB, idx_flat.shape[0]), dtype=np.float16)
    oh[idx_flat, np.arange(idx_flat.shape[0])] = np.float16(1.0)
    return oh


def _in_maps(questions, answers, Eq, Ev, Wa, ba, Wf, bf):
    questions = np.asarray(questions)
    answers = np.asarray(answers)
    w = (questions.astype(np.int64) * 2 + answers.astype(np.int64)) % VOCAB
    pack = np.zeros((128, PC), np.float32)
    pack[0:VOCAB, _EQ:_EQ + DQ] = np.asarray(Eq, np.float32)
    pack[0:VOCAB, _EV:_EV + DV] = np.asarray(Ev, np.float32)
    pack[0:DQ, _WA:_WA + M] = np.asarray(Wa, np.float32)
    wf = np.asarray(Wf, np.float32).reshape(DQ + DV)
    pack[0:DQ, _WFQ] = wf[0:DQ]
    pack[0:DV, _WFR] = wf[DQ:DQ + DV]
    pack[0:VOCAB, _ID:_ID + VOCAB] = np.eye(VOCAB, dtype=np.float32)
    pack[:, _US:_US + 128] = np.triu(np.ones((128, 128), np.float32), k=1)
    pack[0, _ONE:_ONE + VOCAB] = 1.0
    pack[0, _BA:_BA + M] = np.asarray(ba, np.float32).reshape(M)
    pack[0, _BF] = np.asarray(bf, np.float32).reshape(())
    in_maps = []
    for c in range(NCORES):
        sl = slice(c * BS, (c + 1) * BS)
        m = {"pack": pack}
        # token order (b, t): column b*T + t
        m["ohq"] = _onehot(np.ascontiguousarray(questions[:, sl].T).ravel())
        m["ohw"] = _onehot(np.ascontiguousarray(w[:, sl].T).ravel())
        in_maps.append(m)
    return in_maps


def kernel(questions, answers, Eq, Ev, Wa, ba, Wf, bf):
    nc = _get_nc()
    in_maps = _in_maps(questions, answers, Eq, Ev, Wa, ba, Wf, bf)
    res = run_bass_kernel_spmd(nc, in_maps, list(range(NCORES)))
    preds = np.concatenate([res.results[c]["preds"] for c in range(NCORES)], axis=1)
    return preds.astype(np.float32)


# revision 10
# speedup vs baseline: 2.4055x; 1.2935x over previous
"""DKVMN scatter_memory kernel for 8 Trainium2 NeuronCores.

Math: the reference scan only ever uses the (B, M, Dv) memory through
read @ Wf_r, so the whole recurrence collapses to a 32-dim linear
cumulative sum:

  S  = softmax(Eq @ Wa + ba)            (100 x 32)  per-vocab att rows
  cq = Eq @ Wf[:64] + bf                (100,)
  cv = Ev @ Wf[64:]                     (100,)
  w  = (2q + a) % 100
  pred[t,b] = cq[q[t,b]] + sum_{s<t} cv[w[s,b]] * <S[q[t,b]], S[q[s,b]]>

Per core (batch-sharded, Bs=128): the host precomputes fp8 one-hot
encodings of q and w (pure index preprocessing; 0/1 are exact in fp8);
the device gathers S-rows / cq / cv by one-hot matmuls on TensorE
([S|cq] streamed in one 33-col matmul per batch element), then the
cumsum over t is a strict-lower-triangular matmul.  Layout: t on
partitions, (b, m) on free dim.  All parameters arrive in a single
packed f16 tensor; one-hots stream in group-blocked chunks across the
three DMA-capable queues (sync / scalar / gpsimd).
"""
import functools
import numpy as np
import ml_dtypes

import concourse.bass as bass
import concourse.bacc as bacc
import concourse.mybir as mybir
from concourse import tile
from concourse.bass_utils import run_bass_kernel_spmd

T, B, M, DQ, DV, VOCAB = 128, 1024, 32, 64, 64, 100
NCORES = 8
BS = B // NCORES  # 128
N = T * BS        # tokens per core = 16384
GB = 15           # b per group (33*15=495, +15 cv cols = 510 <= 512 psum bank)
GROUPS = [(g * GB, GB) for g in range(8)] + [(120, 8)]
GCOL = [2 * b0 * T for b0, _ in GROUPS]  # ohall column offset of each group
F32 = mybir.dt.float32
F16 = mybir.dt.float16
FP8 = mybir.dt.float8e4
AX = mybir.AxisListType
OP = mybir.AluOpType

# packed-parameter column layout (f16 [128, PC])
_EQ, _EV, _WA, _WFQ, _WFR = 0, 64, 128, 160, 161
_ID, _US, _ONE, _BA, _BF = 162, 262, 390, 490, 522
PC = 523

# one-hot chunk schedule: (queue, [group indices])
CHUNKS = [
    ("sync", [0, 1]),
    ("scalar", [2, 3]),
    ("gpsimd", [4, 5, 6]),
    ("sync", [7, 8]),
]


def _build():
    nc = bacc.Bacc("TRN2", num_devices=NCORES, debug=False, target_bir_lowering=False)
    d = {}
    d["pack"] = nc.dram_tensor("pack", [128, PC], F16, kind="ExternalInput").ap()
    d["ohall"] = nc.dram_tensor("ohall", [VOCAB, 2 * N], FP8, kind="ExternalInput").ap()
    preds = nc.dram_tensor("preds", [T, BS], F32, kind="ExternalOutput").ap()

    with tile.TileContext(nc) as tc:
        with (
            tc.tile_pool(name="sb", bufs=1) as sb,
            tc.tile_pool(name="wk", bufs=3) as wk,
            tc.tile_pool(name="ps", bufs=3, space="PSUM") as ps,
        ):
            P = sb.tile([128, PC], F16)
            nc.scalar.dma_start(P[:], d["pack"][:])
            gtile = [None] * len(GROUPS)
            goff = [0] * len(GROUPS)
            for qname, gids in CHUNKS:
                eng = getattr(nc, qname)
                c0 = GCOL[gids[0]]
                g_end = gids[-1]
                c1 = GCOL[g_end] + 2 * GROUPS[g_end][1] * T
                t_ = sb.tile([VOCAB, c1 - c0], FP8)
                eng.dma_start(t_[:], d["ohall"][:, c0:c1])
                for g in gids:
                    gtile[g] = t_
                    goff[g] = GCOL[g] - c0

            us_t = P[:, _US:_US + 128]

            # ---- parameter tables ----
            p_eqT = ps.tile([DQ, VOCAB], F16, tag="pA")
            p_evT = ps.tile([DV, VOCAB], F16, tag="pP")
            eqT_t = sb.tile([DQ, VOCAB], F16)
            evT_t = sb.tile([DV, VOCAB], F16)
            nc.tensor.transpose(p_eqT[:], P[0:VOCAB, _EQ:_EQ + DQ], P[0:VOCAB, _ID:_ID + VOCAB])
            nc.scalar.copy(eqT_t[:], p_eqT[:])
            nc.tensor.transpose(p_evT[:], P[0:VOCAB, _EV:_EV + DV], P[0:VOCAB, _ID:_ID + VOCAB])
            nc.scalar.copy(evT_t[:], p_evT[:])

            # scat = [softmax(Eq@Wa+ba) | cq]  (100 x 33) fp16
            scat = sb.tile([VOCAB, M + 1], F16)
            p_s = ps.tile([VOCAB, M], F32, tag="pA")
            nc.tensor.matmul(p_s[:], eqT_t[:], P[0:DQ, _WA:_WA + M], start=True, stop=False)
            nc.tensor.matmul(p_s[:], P[0:1, _ONE:_ONE + VOCAB], P[0:1, _BA:_BA + M],
                             start=False, stop=True)
            mx_t = sb.tile([VOCAB, 1], F32)
            sm_t = sb.tile([VOCAB, 1], F32)
            se_t = sb.tile([VOCAB, M], F32)
            nc.vector.tensor_reduce(mx_t[:], p_s[:], AX.X, OP.max)
            nc.vector.tensor_scalar_mul(mx_t[:], mx_t[:], -1.0)
            nc.scalar.activation(se_t[:], p_s[:],
                                 mybir.ActivationFunctionType.Exp,
                                 bias=mx_t[:], scale=1.0)
            nc.vector.tensor_reduce(sm_t[:], se_t[:], AX.X, OP.add)
            nc.vector.reciprocal(sm_t[:], sm_t[:])
            nc.vector.tensor_scalar(out=scat[:, 0:M], in0=se_t[:], scalar1=sm_t[:],
                                    scalar2=None, op0=OP.mult)
            p_cq = ps.tile([VOCAB, 1], F32, tag="pP")
            nc.tensor.matmul(p_cq[:], eqT_t[:], P[0:DQ, _WFQ:_WFQ + 1], start=True, stop=False)
            nc.tensor.matmul(p_cq[:], P[0:1, _ONE:_ONE + VOCAB], P[0:1, _BF:_BF + 1],
                             start=False, stop=True)
            nc.scalar.copy(scat[:, M:M + 1], p_cq[:])

            # cv = Ev @ Wf_r  (100 x 1) fp16
            cv16 = sb.tile([VOCAB, 1], F16)
            p_cv = ps.tile([VOCAB, 1], F32, tag="pA")
            nc.tensor.matmul(p_cv[:], evT_t[:], P[0:DV, _WFR:_WFR + 1], start=True, stop=True)
            nc.scalar.copy(cv16[:], p_cv[:])

            # ---- main pipeline ----
            out_sb = sb.tile([128, BS], F32)
            c_sb = sb.tile([128, BS], F32)

            for gi, (b0, gb) in enumerate(GROUPS):
                oh_g = gtile[gi]
                off = goff[gi]
                woff = off + gb * T
                pA = ps.tile([128, 510], F32, tag="pA")
                pP = ps.tile([128, 480], F32, tag="pP")
                a_g = wk.tile([128, 480], F16, tag="a_sb")
                v_g = wk.tile([128, 480], F16, tag="v_sb")
                ap_g = wk.tile([128, 480], F16, tag="ap_sb")
                cvw_g = wk.tile([128, GB], F16, tag="cvw_sb")
                for k in range(gb):
                    nc.tensor.matmul(pA[:, k * 33:k * 33 + 33],
                                     oh_g[:, off + k * T:off + (k + 1) * T], scat[:],
                                     start=True, stop=True)
                    nc.tensor.matmul(pA[:, 495 + k:496 + k],
                                     oh_g[:, woff + k * T:woff + (k + 1) * T], cv16[:],
                                     start=True, stop=True)
                pA3 = pA[:, 0:gb * 33].rearrange("p (k c) -> p k c", c=33)
                # cq column -> c_sb ; cv row -> f16 ; A rows -> f16
                nc.scalar.copy(c_sb[:, b0:b0 + gb], pA3[:, :, M:M + 1])
                nc.scalar.copy(cvw_g[:, 0:gb], pA[:, 495:495 + gb])
                nc.scalar.copy(a_g[:, 0:gb * M].rearrange("p (k c) -> p k c", c=M),
                               pA3[:, :, 0:M])
                # v = A * cv[w] (broadcast cvw along m)
                a3 = a_g[:, 0:gb * M].rearrange("p (k c) -> p k c", c=M)
                cvb = cvw_g[:, 0:gb].rearrange("p (k c) -> p k c", c=1)
                a3b, cvb = bass.broadcast_tensor_aps(a3, cvb)
                nc.vector.tensor_tensor(
                    v_g[:, 0:gb * M].rearrange("p (k c) -> p k c", c=M),
                    a3b, cvb, OP.mult)
                # exclusive cumsum over t (strict upper as lhsT)
                nc.tensor.matmul(pP[:, 0:gb * M], us_t, v_g[:, 0:gb * M],
                                 start=True, stop=True)
                # pred contribution: sum_m A * C
                nc.vector.tensor_tensor(
                    ap_g[:, 0:gb * M], a_g[:, 0:gb * M], pP[:, 0:gb * M], OP.mult)
                nc.vector.tensor_reduce(
                    out_sb[:, b0:b0 + gb],
                    ap_g[:, 0:gb * M].rearrange("p (b m) -> p b m", m=M),
                    AX.X, OP.add)

            nc.vector.tensor_add(out_sb[:], out_sb[:], c_sb[:])
            nc.sync.dma_start(preds[:], out_sb[:])

    nc.compile()
    return nc


@functools.lru_cache(maxsize=1)
def _get_nc():
    return _build()


def _in_maps(questions, answers, Eq, Ev, Wa, ba, Wf, bf):
    questions = np.asarray(questions)
    answers = np.asarray(answers)
    w = (questions.astype(np.int64) * 2 + answers.astype(np.int64)) % VOCAB
    pack = np.zeros((128, PC), np.float16)
    pack[0:VOCAB, _EQ:_EQ + DQ] = np.asarray(Eq, np.float32)
    pack[0:VOCAB, _EV:_EV + DV] = np.asarray(Ev, np.float32)
    pack[0:DQ, _WA:_WA + M] = np.asarray(Wa, np.float32)
    wf = np.asarray(Wf, np.float32).reshape(DQ + DV)
    pack[0:DQ, _WFQ] = wf[0:DQ]
    pack[0:DV, _WFR] = wf[DQ:DQ + DV]
    pack[0:VOCAB, _ID:_ID + VOCAB] = np.eye(VOCAB, dtype=np.float16)
    pack[:, _US:_US + 128] = np.triu(np.ones((128, 128), np.float16), k=1)
    pack[0, _ONE:_ONE + VOCAB] = 1.0
    pack[0, _BA:_BA + M] = np.asarray(ba, np.float32).reshape(M)
    pack[0, _BF] = np.asarray(bf, np.float32).reshape(())
    in_maps = []
    ar = np.arange(BS * T)
    for c in range(NCORES):
        sl = slice(c * BS, (c + 1) * BS)
        qf = np.ascontiguousarray(questions[:, sl].T).ravel()
        wfl = np.ascontiguousarray(w[:, sl].T).ravel()
        # group-blocked: [q cols | w cols] per group of GB batch elements
        oh = np.zeros((VOCAB, 2 * N), dtype=ml_dtypes.float8_e4m3)
        for gi, (b0, gb) in enumerate(GROUPS):
            base = GCOL[gi]
            tok = slice(b0 * T, (b0 + gb) * T)
            nt = (tok.stop - tok.start)
            oh[qf[tok], base + np.arange(nt)] = 1.0
            oh[wfl[tok], base + nt + np.arange(nt)] = 1.0
        in_maps.append({"pack": pack, "ohall": oh})
    return in_maps


def kernel(questions, answers, Eq, Ev, Wa, ba, Wf, bf):
    nc = _get_nc()
    in_maps = _in_maps(questions, answers, Eq, Ev, Wa, ba, Wf, bf)
    res = run_bass_kernel_spmd(nc, in_maps, list(range(NCORES)))
    preds = np.concatenate([res.results[c]["preds"] for c in range(NCORES)], axis=1)
    return preds.astype(np.float32)
